# revision 2
# baseline (speedup 1.0000x reference)
"""MoE kernel for Trainium2, expert-parallel across 8 NeuronCores. v2.

Problem (hardcoded): E=8 experts, top_k=2, H=1024, F=4096, B=2, S=2048
(T=4096 tokens). Expert c lives on core c. Each core:
  1. computes router logits for ALL tokens locally (bf16, rw stationary),
  2. top-2 mask + softmax weight for its expert; compacts BOTH the selected
     token ids and their combine weights with two gpsimd sparse_gathers
     (identical scan order), producing int16 index tiles for the bulk
     SWDGE ops and a slot-major weight vector,
  3. one bulk dma_gather pulls the selected tokens' fp8 rows, PE-transposes
     them to fp8 xcT, runs up-proj -> gelu -> down-proj in fp8 DoubleRow
     (weights pre-scaled x64 on host), scales rows by the combine weight
     into a contiguous [128, 9*512] staging tile per H-half, and lands them
     with one dma_scatter_add per half into a zeroed [T, 512] bf16 buffer,
  4. ReduceScatter sums expert contributions across cores (one per H-half);
     the shared expert runs in fp8 DoubleRow for the core's 512-token slice
     (sh_down_b folded into x_slice on the host) and
     out_slice = x_slice' + shared + expert_sum.
Host assembles the 8 slices into the full [B, S, H] output.
"""

import numpy as np
import ml_dtypes

import concourse.bacc as bacc
import concourse.mybir as mybir
import concourse.tile as tile
from concourse import bass
from concourse.bass_utils import run_bass_kernel_spmd
from concourse.masks import make_identity

N_CORES = 8
T = 4096          # tokens
H = 1024          # hidden
F = 4096          # expert hidden
E = 8             # experts
P = 128
TT = T // P       # 32 token tiles
C = 1152          # per-expert token capacity (max actual count ~1086)
CT = C // P       # 9 capacity tiles
SL = T // N_CORES  # 512 tokens owned per core
BIG = 1.0e6       # pad sentinel (> any token id, survives sparse_gather)
WS = 64.0         # fp8 weight scale
NA = 12           # shared-up tiles computed early (before expert phase)

FP = mybir.dt.float32
BF = mybir.dt.bfloat16
F8 = mybir.dt.float8e4
I16 = mybir.dt.int16
DRM = mybir.MatmulPerfMode.DoubleRow
KT = H // P       # 8 contraction tiles
FT = F // P       # 32 expert-hidden tiles


class _NullCtx:
    def __enter__(self):
        return None

    def __exit__(self, *a):
        return False


def build(with_rs=True, loop_n=0, skip_wdma=False, skip_zero=False,
          skip_gather=False, skip_scat=False, skip_sg=False):
    nc = bacc.Bacc("TRN2", target_bir_lowering=False, debug=False,
                   num_devices=N_CORES)

    # ---- I/O ----
    xTr = nc.dram_tensor("xTr", [H, T], BF, kind="ExternalInput")
    xTbs = nc.dram_tensor("xTbs", [H, SL], BF, kind="ExternalInput")
    x_slice = nc.dram_tensor("x_slice", [SL, H], FP, kind="ExternalInput")
    xb8 = nc.dram_tensor("xb8", [T, H], F8, kind="ExternalInput")
    rwb = nc.dram_tensor("rwb", [H, E], BF, kind="ExternalInput")
    rbb = nc.dram_tensor("rbb", [P, E], FP, kind="ExternalInput")
    eselb = nc.dram_tensor("eselb", [P, E], FP, kind="ExternalInput")
    upw8 = nc.dram_tensor("upw8", [F, H], F8, kind="ExternalInput")  # swizzled, xWS
    upb = nc.dram_tensor("upb", [P, FT], FP, kind="ExternalInput")
    dww8 = nc.dram_tensor("dww8", [F, H], F8, kind="ExternalInput")  # xWS
    dwb = nc.dram_tensor("dwb", [1, H], FP, kind="ExternalInput")
    suw = nc.dram_tensor("suw", [F, H], BF, kind="ExternalInput")  # swizzled
    sub = nc.dram_tensor("sub", [P, FT], FP, kind="ExternalInput")
    sdw = nc.dram_tensor("sdw", [F, H], BF, kind="ExternalInput")
    tokid1 = nc.dram_tensor("tokid1", [P, TT], FP, kind="ExternalInput")
    rep16 = nc.dram_tensor("rep16", [16, P], FP, kind="ExternalInput")
    out_slice = nc.dram_tensor("out_slice", [SL, H], FP, kind="ExternalOutput")

    with tile.TileContext(nc) as tc:
        with (
            tc.tile_pool(name="const", bufs=1) as cpool,
            tc.tile_pool(name="sbig", bufs=1) as sbig,
            tc.tile_pool(name="sio", bufs=3) as sio,
            tc.tile_pool(name="wpool", bufs=3) as wpool,
            tc.tile_pool(name="small", bufs=1) as small,
            tc.tile_pool(name="psum", bufs=1, space="PSUM") as psum,
            tc.tile_pool(name="dram", bufs=1, space="DRAM") as dram,
        ):
            # ---- internal DRAM ----
            vvals = dram.tile([P, TT], FP)    # token-or-minus-one, contig
            wvals = dram.tile([P, TT], FP)    # weight-or-minus-one, contig
            wd = dram.tile([C, 1], FP)        # slot-major combine weights
            scatA = dram.tile([T, 512], BF)
            scatB = dram.tile([T, 512], BF)
            rsA = dram.tile([SL, 512], BF)
            rsB = dram.tile([SL, 512], BF)

            # ---- constants ----
            id_b = cpool.tile([P, P], BF)
            make_identity(nc, id_b[:])
            id_8 = cpool.tile([P, P], F8)
            nc.vector.tensor_copy(id_8[:], id_b[:])
            rbb_sb = cpool.tile([P, E], FP)
            nc.sync.dma_start(out=rbb_sb[:], in_=rbb[:])
            eselb_sb = cpool.tile([P, E], FP)
            nc.sync.dma_start(out=eselb_sb[:], in_=eselb[:])
            tok_sb = cpool.tile([P, TT], FP)
            nc.sync.dma_start(out=tok_sb[:], in_=tokid1[:])
            upb_sb = cpool.tile([P, FT], FP)
            nc.sync.dma_start(out=upb_sb[:], in_=upb[:])
            sub_sb = cpool.tile([P, FT], FP)
            nc.sync.dma_start(out=sub_sb[:], in_=sub[:])
            dwb_row = cpool.tile([1, H], FP)
            nc.sync.dma_start(out=dwb_row[:], in_=dwb[:])
            rep16_sb = cpool.tile([16, P], FP)
            nc.sync.dma_start(out=rep16_sb[:], in_=rep16[:])
            ws_row = cpool.tile([1, P], FP)
            nc.vector.memset(ws_row[:], WS)
            zero_big = cpool.tile([P, 1024], BF)
            nc.vector.memset(zero_big[:], 0.0)
            id_f = cpool.tile([P, P], FP)
            make_identity(nc, id_f[:])
            # router weights: [H, E] -> [128, (k e)]
            rw_sb = cpool.tile([P, KT * E], BF)
            nc.sync.dma_start(
                out=rw_sb[:].rearrange("p (k e) -> p k e", e=E),
                in_=rwb[:, :].rearrange("(k p) e -> p k e", p=P))

            # broadcast down-proj bias across partitions via K=1 matmul
            # (scaled by WS to match the fp8-scaled PSUM values)
            dwb_b = cpool.tile([P, H], FP)
            for hck in range(2):
                pb = psum.tile([P, 512], FP, tag="ptp", bufs=1)
                nc.tensor.matmul(
                    out=pb[:], lhsT=ws_row[:],
                    rhs=dwb_row[:, 512 * hck:512 * (hck + 1)],
                    start=True, stop=True)
                nc.vector.tensor_copy(dwb_b[:, 512 * hck:512 * (hck + 1)],
                                      pb[:])

            def body():
                # ---- phase B: local expert-major router (bf16 x bf16) ----
                pt = psum.tile([P, E * TT], FP, tag="ptp", bufs=1)

                def btran(ch, lgc):
                    for jl in range(4):
                        j = 4 * ch + jl
                        nc.tensor.transpose(
                            out=pt[:, E * j:E * (j + 1)],
                            in_=lgc[:, P * jl:P * (jl + 1)],
                            identity=id_f[:E, :E])

                lgcs = []
                for ch in range(KT):
                    xth = []
                    for h2 in range(2):
                        xt = sio.tile([P, 4 * 512], BF, tag="xrt", bufs=3,
                                      name=f"xt{ch}_{h2}")
                        nc.sync.dma_start(
                            out=xt[:],
                            in_=xTr[P * ch:P * (ch + 1),
                                    2048 * h2:2048 * (h2 + 1)])
                        xth.append(xt)
                    pl = psum.tile([E, 512], FP, tag="pu", bufs=3,
                                   name=f"plr{ch}")
                    for k in range(KT):
                        nc.tensor.matmul(
                            out=pl[:],
                            lhsT=rw_sb[:, E * k:E * (k + 1)],
                            rhs=xth[k // 4][:, 512 * (k % 4):
                                            512 * (k % 4 + 1)],
                            start=(k == 0), stop=(k == KT - 1))
                    lgc = small.tile([E, 512], FP, tag="lgc", bufs=3,
                                     name=f"lgc{ch}")
                    nc.vector.tensor_copy(lgc[:], pl[:])
                    lgcs.append(lgc)
                    if ch >= 2:
                        btran(ch - 2, lgcs[ch - 2])

                # shared-up input; issue early so it lands before G1
                xsh = sbig.tile([P, KT * SL], BF)
                for k in range(KT):
                    nc.scalar.dma_start(out=xsh[:, SL * k:SL * (k + 1)],
                                        in_=xTbs[P * k:P * (k + 1), :])
                # prefetch the residual slice for phase H
                xss = [sbig.tile([P, H], FP, name=f"xss{i}")
                       for i in range(SL // P)]
                for i in range(SL // P):
                    nc.scalar.dma_start(out=xss[i][:],
                                        in_=x_slice[P * i:P * (i + 1), :])

                def zero_scat():
                    # zero the scatter-add targets; 256 rows per DMA
                    if not skip_zero:
                        for j in range(T // 256):
                            for buf in (scatA, scatB):
                                nc.sync.dma_start(
                                    out=buf[256 * j:256 * (j + 1), :]
                                    .rearrange("(p t) c -> p (t c)", t=2),
                                    in_=zero_big[:])

                zero_scat()

                # remaining router transposes
                btran(KT - 2, lgcs[KT - 2])
                btran(KT - 1, lgcs[KT - 1])
                lg = sbig.tile([P, E * TT], FP)
                rbb_bc = rbb_sb[:].rearrange(
                    "p (o e) -> p o e", o=1).to_broadcast([P, TT, E])
                nc.vector.tensor_tensor(
                    out=lg[:].rearrange("p (j e) -> p j e", e=E),
                    in0=pt[:].rearrange("p (j e) -> p j e", e=E),
                    in1=rbb_bc, op=mybir.AluOpType.add)

                # ---- phase C: top-2 mask, my softmax weight (fp32) ----
                lg8 = lg[:].rearrange("p (j e) -> p j e", e=E)
                esel_bc = eselb_sb[:].rearrange(
                    "p (o e) -> p o e", o=1).to_broadcast([P, TT, E])
                sel = small.tile([P, E * TT], FP, bufs=1)
                nc.vector.tensor_tensor(
                    out=sel[:].rearrange("p (j e) -> p j e", e=E),
                    in0=lg8, in1=esel_bc, op=mybir.AluOpType.mult)
                lmy = small.tile([P, TT], FP)
                nc.vector.tensor_reduce(
                    lmy[:], sel[:].rearrange("p (j e) -> p j e", e=E),
                    axis=mybir.AxisListType.X, op=mybir.AluOpType.add)
                m1 = small.tile([P, TT], FP)
                nc.vector.tensor_reduce(m1[:], lg8, axis=mybir.AxisListType.X,
                                        op=mybir.AluOpType.max)
                m1b = m1[:].rearrange("p (j o) -> p j o", o=1).to_broadcast(
                    [P, TT, E])
                lmyb = lmy[:].rearrange("p (j o) -> p j o", o=1).to_broadcast(
                    [P, TT, E])
                gtm = small.tile([P, E * TT], FP, bufs=1)
                nc.vector.tensor_tensor(
                    out=gtm[:].rearrange("p (j e) -> p j e", e=E),
                    in0=lg8, in1=lmyb, op=mybir.AluOpType.is_gt)
                cnt = small.tile([P, TT], FP)
                nc.vector.tensor_reduce(
                    cnt[:], gtm[:].rearrange("p (j e) -> p j e", e=E),
                    axis=mybir.AxisListType.X, op=mybir.AluOpType.add)
                mask0 = small.tile([P, TT], FP)
                nc.vector.tensor_scalar(mask0[:], cnt[:], 1.5, None,
                                        op0=mybir.AluOpType.is_le)
                ex = sel  # sel is dead after lmy; reuse its buffer
                nc.vector.tensor_tensor(
                    out=ex[:].rearrange("p (j e) -> p j e", e=E),
                    in0=lg8, in1=m1b, op=mybir.AluOpType.subtract)
                nc.scalar.activation(ex[:], ex[:],
                                     mybir.ActivationFunctionType.Exp)
                ssum = small.tile([P, TT], FP)
                nc.vector.tensor_reduce(
                    ssum[:], ex[:].rearrange("p (j e) -> p j e", e=E),
                    axis=mybir.AxisListType.X, op=mybir.AluOpType.add)
                rcp = small.tile([P, TT], FP)
                nc.vector.reciprocal(rcp[:], ssum[:])
                tmy = small.tile([P, TT], FP)
                nc.vector.tensor_tensor(out=tmy[:], in0=lmy[:], in1=m1[:],
                                        op=mybir.AluOpType.subtract)
                nc.scalar.activation(tmy[:], tmy[:],
                                     mybir.ActivationFunctionType.Exp)
                w0 = small.tile([P, TT], FP)
                nc.vector.tensor_tensor(out=w0[:], in0=tmy[:], in1=rcp[:],
                                        op=mybir.AluOpType.mult)
                # pre-divide the combine weight by WS (fp8 weight scale)
                nc.vector.tensor_scalar(w0[:], w0[:], 1.0 / WS, None,
                                        op0=mybir.AluOpType.mult)
                # v = tokid1 * mask0 - 1  (token id if selected else -1)
                vv = small.tile([P, TT], FP)
                nc.vector.tensor_tensor(out=vv[:], in0=tok_sb[:], in1=mask0[:],
                                        op=mybir.AluOpType.mult)
                nc.vector.tensor_scalar_add(vv[:], vv[:], -1.0)
                # vw = (w0 + 1) * mask0 - 1  (weight if selected else -1)
                vw = small.tile([P, TT], FP)
                nc.vector.tensor_scalar_add(vw[:], w0[:], 1.0)
                nc.vector.tensor_tensor(out=vw[:], in0=vw[:], in1=mask0[:],
                                        op=mybir.AluOpType.mult)
                nc.vector.tensor_scalar_add(vw[:], vw[:], -1.0)
                # contiguous stores (same bijection for both)
                nc.scalar.dma_start(out=vvals[:, :], in_=vv[:])
                nc.scalar.dma_start(out=wvals[:, :], in_=vw[:])

                # ---- phase D: compact ids + weights via sparse_gather ----
                NPAD = C // 16
                vsb = small.tile([16, T // 16 + NPAD], FP)
                nc.vector.memset(vsb[:], BIG)
                nc.scalar.dma_start(
                    out=vsb[:, :T // 16],
                    in_=vvals[:, :].rearrange("(q r) j -> q (r j)", r=8))
                wsb = small.tile([16, T // 16 + NPAD], FP)
                nc.vector.memset(wsb[:], 0.0)  # pad weights -> 0
                nc.scalar.dma_start(
                    out=wsb[:, :T // 16],
                    in_=wvals[:, :].rearrange("(q r) j -> q (r j)", r=8))
                gout = small.tile([16, C // 16], FP)
                wout = small.tile([16, C // 16], FP)
                if skip_sg:
                    nc.vector.memset(gout[:], 5.0)
                    nc.vector.memset(wout[:], 0.001)
                else:
                    ng = small.tile([1, 1], mybir.dt.uint32)
                    nc.gpsimd.sparse_gather(out=gout[:], in_=vsb[:],
                                            num_found=ng[:])
                    ng2 = small.tile([1, 1], mybir.dt.uint32)
                    nc.gpsimd.sparse_gather(out=wout[:], in_=wsb[:],
                                            num_found=ng2[:])
                # index values: pads -> 0 for the gather, -1 for the scatter
                selm = small.tile([16, C // 16], FP)
                nc.vector.tensor_scalar(selm[:], gout[:], float(T), None,
                                        op0=mybir.AluOpType.is_lt)
                idgf = small.tile([16, C // 16], FP)
                nc.vector.tensor_tensor(out=idgf[:], in0=gout[:], in1=selm[:],
                                        op=mybir.AluOpType.mult)
                # slot-major combine weights -> [P, CT]
                nc.scalar.dma_start(
                    out=wd[:, 0].rearrange("(f q) -> q f", q=16), in_=wout[:])
                wc_all = cpool.tile([P, CT], FP, name="wc_all")
                nc.scalar.dma_start(
                    out=wc_all[:],
                    in_=wd[:, 0].rearrange("(i p) -> p i", p=P))

                # ---- phase G1: shared expert up-proj (bf16), NA tiles ----
                sgt = sbig.tile([P, FT * SL], BF)

                def shared_up(ft):
                    pu = psum.tile([P, 512], FP, tag="pu", bufs=3,
                                   name=f"psh{ft}")
                    uw = wpool.tile([P, KT * P], BF, tag="suw", bufs=4,
                                    name=f"suw{ft}")
                    nc.sync.dma_start(out=uw[:],
                                      in_=suw[P * ft:P * (ft + 1), :])
                    for k in range(KT):
                        nc.tensor.matmul(out=pu[:],
                                         lhsT=uw[:, P * k:P * (k + 1)],
                                         rhs=xsh[:, SL * k:SL * (k + 1)],
                                         start=(k == 0), stop=(k == KT - 1))
                    nc.scalar.activation(
                        sgt[:, SL * ft:SL * (ft + 1)], pu[:],
                        mybir.ActivationFunctionType.Gelu,
                        bias=sub_sb[:, ft:ft + 1])

                for ft in range(4):
                    shared_up(ft)

                # replicate idx values to all 8 gpsimd core groups (the SWDGE
                # ucode reads indexes from its own 16 partitions) via a
                # one-hot [16, 128] broadcast matmul, then convert to int16.
                # pads point at token 0; their rows carry weight 0 so the
                # scatter-add contributes nothing.
                idx16g = small.tile([P, C // 16], I16, bufs=1)
                pr = psum.tile([P, C // 16], FP, tag="ptp", bufs=1,
                               name="pr_g")
                nc.tensor.matmul(out=pr[:], lhsT=rep16_sb[:],
                                 rhs=idgf[:], start=True, stop=True)
                nc.vector.tensor_copy(idx16g[:], pr[:])

                # ---- phase E: bulk-gather token rows, transpose to fp8 ----
                # (split at 512 slots: the SWDGE ring holds 1024 descriptors;
                # issued now so the gather runs under the rest of G1)
                xg8 = sbig.tile([P, CT * H], F8)
                xg8sp = xg8[:].rearrange("p (i h) -> p i h", h=H)
                if skip_gather:
                    for i in range(CT):
                        nc.sync.dma_start(out=xg8sp[:, i, :],
                                          in_=xb8[P * i:P * (i + 1), :])
                else:
                    nc.gpsimd.dma_gather(
                        out_ap=xg8sp[:, :4, :],
                        in_ap=xb8[:, :], idxs_ap=idx16g[:, :32],
                        num_idxs=512, num_idxs_reg=512, elem_size=H)
                    nc.gpsimd.dma_gather(
                        out_ap=xg8sp[:, 4:, :],
                        in_ap=xb8[:, :], idxs_ap=idx16g[:, 32:],
                        num_idxs=C - 512, num_idxs_reg=C - 512, elem_size=H)

                for ft in range(4, NA):
                    shared_up(ft)

                xg8v = xg8[:].rearrange("p (i h) -> p i h", h=H)
                xcT8 = sbig.tile([P, KT * C], F8)
                for i in range(CT):
                    xgb = sio.tile([P, H], BF, tag="xgb", bufs=3,
                                   name=f"xgb{i}")
                    nc.scalar.activation(xgb[:], xg8v[:, i, :],
                                         mybir.ActivationFunctionType.Copy)
                    px = psum.tile([P, KT * P], BF, tag="pu", bufs=3,
                                   name=f"px{i}")
                    for k in range(KT):
                        nc.tensor.transpose(out=px[:, P * k:P * (k + 1)],
                                            in_=xgb[:, P * k:P * (k + 1)],
                                            identity=id_b[:])
                    nc.vector.tensor_copy(
                        xcT8[:].rearrange("p (k c) -> p k c", c=C)[
                            :, :, P * i:P * (i + 1)],
                        px[:].rearrange("p (k c) -> p k c", c=P))

                # ---- phase F: fp8 DoubleRow up-proj -> gelu -> down-proj
                # -> staged scatter-add, H-half-major for early RS ----
                ugt8 = sbig.tile([P, FT * C], F8)
                xcT8v = xcT8[:].rearrange("p (k c) -> p k c", c=C)
                for ft in range(FT):
                    uw8 = wpool.tile([P, KT * P], F8, tag="uw8", bufs=4,
                                     name=f"uw{ft}")
                    if skip_wdma:
                        nc.vector.memset(uw8[:], 0.5)
                    else:
                        nc.sync.dma_start(
                            out=uw8[:], in_=upw8[P * ft:P * (ft + 1), :])
                    uw8v = uw8[:].rearrange("p (k q) -> p k q", q=P)
                    CH3 = ((0, 512), (512, 512), (1024, C - 1024))
                    pus = [psum.tile([P, 512], FP, tag="pu", bufs=3,
                                     name=f"pu{ft}_{soff}")
                           if c3 < 2 else
                           psum.tile([P, 512], FP, tag="ptp", bufs=1,
                                     name=f"pu{ft}_{soff}")
                           for c3, (soff, slen) in enumerate(CH3)]
                    # kp outer so each DoubleRow LDWEIGHTS serves all three
                    # column chunks (4 LDWs per ft instead of 12)
                    for kp in range(KT // 2):
                        for c3, (soff, slen) in enumerate(CH3):
                            nc.tensor.matmul(
                                out=pus[c3][:, :slen],
                                lhsT=uw8v[:, 2 * kp:2 * kp + 2, :],
                                rhs=xcT8v[:, 2 * kp:2 * kp + 2,
                                          soff:soff + slen],
                                start=(kp == 0), stop=(kp == KT // 2 - 1),
                                perf_mode=DRM)
                    for c3, (soff, slen) in enumerate(CH3):
                        nc.scalar.activation(
                            ugt8[:, C * ft + soff:C * ft + soff + slen],
                            pus[c3][:, :slen],
                            mybir.ActivationFunctionType.Gelu,
                            scale=1.0 / WS, bias=upb_sb[:, ft:ft + 1])

                ugt8v = ugt8[:].rearrange("p (f c) -> p f c", c=C)
                gap_ft = [NA]  # shared-up tiles slotted into batch drains
                ysb = sbig.tile([P, CT * 512], BF, name="ysb")
                for hc in range(2):
                    for (b0, b1) in ((0, 4), (4, CT)):
                        # batch 2 carries 5 tiles: 4 on the pd ring plus one
                        # on the otherwise-idle ptp bank
                        pds = [psum.tile([P, 512], FP, tag="pd", bufs=4,
                                         name=f"pd{hc}_{i}")
                               if i < b0 + 4 else
                               psum.tile([P, 512], FP, tag="ptp", bufs=1,
                                         name=f"pd{hc}_{i}")
                               for i in range(b0, b1)]
                        for fp_ in range(FT // 2):
                            dw2 = wpool.tile([P, 2 * 512], F8, tag="dw8",
                                             bufs=6, name=f"dw{hc}_{fp_}")
                            if skip_wdma:
                                nc.vector.memset(dw2[:], 0.5)
                            else:
                                nc.sync.dma_start(
                                    out=dw2[:].rearrange(
                                        "p (two h) -> p two h", two=2),
                                    in_=dww8[256 * fp_:256 * (fp_ + 1),
                                             512 * hc:512 * (hc + 1)]
                                    .rearrange("(two p) h -> p two h", p=P))
                            dw2v = dw2[:].rearrange("p (two h) -> p two h",
                                                    two=2)
                            for i in range(b0, b1):
                                nc.tensor.matmul(
                                    out=pds[i - b0][:],
                                    lhsT=ugt8v[:, 2 * fp_:2 * fp_ + 2,
                                               P * i:P * (i + 1)],
                                    rhs=dw2v,
                                    start=(fp_ == 0),
                                    stop=(fp_ == FT // 2 - 1),
                                    perf_mode=DRM)
                        for i in range(b0, b1):
                            ys = ysb[:, 512 * i:512 * (i + 1)]
                            nc.vector.tensor_add(
                                ys, pds[i - b0][:],
                                dwb_b[:, 512 * hc:512 * (hc + 1)])
                            nc.vector.tensor_scalar_mul(
                                ys, ys, wc_all[:, i:i + 1])
                        # independent PE work to cover the batch's
                        # accumulation-group drain
                        shared_up(gap_ft[0])
                        gap_ft[0] += 1
                        # scatter this batch while the next one computes
                        scat = scatA if hc == 0 else scatB
                        ysbv = ysb[:].rearrange("p (i c) -> p i c", c=512)
                        if skip_scat:
                            for i in range(b0, b1):
                                if P * (i + 1) <= T:
                                    nc.sync.dma_start(
                                        out=scat[P * i:P * (i + 1), :],
                                        in_=ysbv[:, i, :])
                        elif b0 == 0:
                            nc.gpsimd.dma_scatter_add(
                                scat[:, :], ysbv[:, :4, :],
                                idx16g[:, :32], 512, 512, 512)
                        else:
                            nc.gpsimd.dma_scatter_add(
                                scat[:, :], ysbv[:, 4:, :],
                                idx16g[:, 32:], C - 512, C - 512, 512)
                    src, dst = (scatA, rsA) if hc == 0 else (scatB, rsB)
                    if with_rs:
                        nc.gpsimd.collective_compute(
                            "ReduceScatter", mybir.AluOpType.add,
                            replica_groups=[list(range(N_CORES))],
                            ins=[src[:]], outs=[dst[:]])
                    else:
                        nc.sync.dma_start(out=dst[:], in_=src[:SL, :])

                # ---- phase G2: rest of shared expert (hides the RS tail)
                for ft in range(gap_ft[0], FT):
                    shared_up(ft)
                sho = [sbig.tile([P, H], FP, name=f"sho{i}")
                       for i in range(SL // P)]
                for hc in range(2):
                    pdsh = [psum.tile([P, 512], FP, tag="pd", bufs=4,
                                      name=f"pds{hc}_{i}")
                            for i in range(SL // P)]
                    for ft in range(FT):
                        dwt = wpool.tile([P, 512], BF, tag="sdw", bufs=6,
                                         name=f"sdw{hc}_{ft}")
                        nc.sync.dma_start(
                            out=dwt[:],
                            in_=sdw[P * ft:P * (ft + 1),
                                    512 * hc:512 * (hc + 1)])
                        for i in range(SL // P):
                            nc.tensor.matmul(
                                out=pdsh[i][:],
                                lhsT=sgt[:, SL * ft + P * i:
                                         SL * ft + P * (i + 1)],
                                rhs=dwt[:],
                                start=(ft == 0), stop=(ft == FT - 1))
                    for i in range(SL // P):
                        nc.vector.tensor_copy(
                            sho[i][:, 512 * hc:512 * (hc + 1)],
                            pdsh[i][:])

                # ---- phase H: out = x_slice' + shared + expert sum ----
                for i in range(SL // P):
                    rsl = sio.tile([P, H], BF, tag="rsl", bufs=2)
                    nc.sync.dma_start(out=rsl[:, :512],
                                      in_=rsA[P * i:P * (i + 1), :])
                    nc.sync.dma_start(out=rsl[:, 512:],
                                      in_=rsB[P * i:P * (i + 1), :])
                    nc.vector.tensor_add(xss[i][:], xss[i][:], rsl[:])
                    nc.vector.tensor_add(xss[i][:], xss[i][:], sho[i][:])
                    nc.sync.dma_start(out=out_slice[P * i:P * (i + 1), :],
                                      in_=xss[i][:])

            with (tc.For_i(0, loop_n, 1) if loop_n else _NullCtx()):
                body()
    nc.finalize()
    return nc


_NC_CACHE = None


def _get_nc():
    global _NC_CACHE
    if _NC_CACHE is None:
        _NC_CACHE = build()
    return _NC_CACHE


def _swizzle(w):
    # [H, F] -> [F, H] with row = ft*128 + h%128, col = (h//128)*128 + f%128
    return np.ascontiguousarray(
        w.reshape(H // P, P, F // P, P).transpose(2, 1, 0, 3).reshape(F, H))


def make_in_maps(inputs):
    x = np.asarray(inputs["hidden_states"], dtype=np.float32).reshape(T, H)
    router_w = np.asarray(inputs["router_w"], dtype=np.float32)
    router_b = np.asarray(inputs["router_b"], dtype=np.float32)
    up_w = np.asarray(inputs["up_w"], dtype=np.float32)
    up_b = np.asarray(inputs["up_b"], dtype=np.float32)
    down_w = np.asarray(inputs["down_w"], dtype=np.float32)
    down_b = np.asarray(inputs["down_b"], dtype=np.float32)
    sh_up_w = np.asarray(inputs["sh_up_w"], dtype=np.float32)
    sh_up_b = np.asarray(inputs["sh_up_b"], dtype=np.float32)
    sh_down_w = np.asarray(inputs["sh_down_w"], dtype=np.float32)
    sh_down_b = np.asarray(inputs["sh_down_b"], dtype=np.float32)

    bf = ml_dtypes.bfloat16
    f8 = ml_dtypes.float8_e4m3

    def q8(a):
        return np.ascontiguousarray(np.clip(a * WS, -240, 240).astype(f8))

    def q8x(a):
        return np.ascontiguousarray(np.clip(a, -240, 240).astype(f8))

    xT = np.ascontiguousarray(x.T)
    # router chunk-contiguous layout
    xTr_ = np.ascontiguousarray(
        xT.reshape(KT, P, T // 512, 512).transpose(2, 1, 0, 3)
        .reshape(H, T).astype(bf))
    xb8_ = q8x(x)
    rwb_ = np.ascontiguousarray(router_w.astype(bf))
    rbb_ = np.ascontiguousarray(np.tile(router_b.reshape(1, E), (P, 1)))
    tokid1 = (np.arange(P)[:, None] + P * np.arange(TT)[None, :] + 1.0).astype(
        np.float32)
    suw_ = np.ascontiguousarray(_swizzle(sh_up_w).astype(bf))
    sub_ = np.ascontiguousarray(sh_up_b.reshape(FT, P).T.astype(np.float32))
    sdw_ = np.ascontiguousarray(sh_down_w.astype(bf))
    eye = np.eye(E, dtype=np.float32)
    rep16_ = np.ascontiguousarray(
        np.tile(np.eye(16, dtype=np.float32), (1, P // 16)).reshape(16, P))

    in_maps = []
    for c in range(N_CORES):
        in_maps.append({
            "xTr": xTr_,
            "xTbs": np.ascontiguousarray(
                xT[:, SL * c:SL * (c + 1)].astype(bf)),
            "x_slice": np.ascontiguousarray(
                x[SL * c:SL * (c + 1)] + sh_down_b.reshape(1, H)),
            "xb8": xb8_,
            "rwb": rwb_,
            "rbb": rbb_,
            "eselb": np.ascontiguousarray(np.tile(eye[c:c + 1], (P, 1))),
            "upw8": q8(_swizzle(up_w[c])),
            "upb": np.ascontiguousarray(
                up_b[c].reshape(FT, P).T.astype(np.float32)),
            "dww8": q8(down_w[c]),
            "dwb": down_b[c].reshape(1, H).astype(np.float32),
            "suw": suw_, "sub": sub_, "sdw": sdw_,
            "tokid1": tokid1, "rep16": rep16_,
        })
    return in_maps


def assemble(results):
    out = np.concatenate([results[c]["out_slice"] for c in range(N_CORES)],
                         axis=0)
    return out.reshape(2, 2048, H).astype(np.float32)


def kernel(**inputs):
    nc = _get_nc()
    in_maps = make_in_maps(inputs)
    res = run_bass_kernel_spmd(nc, in_maps, core_ids=list(range(N_CORES)))
    return assemble(res.results)


# revision 3
# speedup vs baseline: 1.1409x; 1.1409x over previous
"""MoE kernel for Trainium2, expert-parallel across 8 NeuronCores. v2.

Problem (hardcoded): E=8 experts, top_k=2, H=1024, F=4096, B=2, S=2048
(T=4096 tokens). Expert c lives on core c. Each core:
  1. computes router logits for ALL tokens locally (bf16, rw stationary),
  2. top-2 mask + softmax weight for its expert; compacts BOTH the selected
     token ids and their combine weights with two gpsimd sparse_gathers
     (identical scan order), producing int16 index tiles for the bulk
     SWDGE ops and a slot-major weight vector,
  3. one bulk dma_gather pulls the selected tokens' fp8 rows, PE-transposes
     them to fp8 xcT, runs up-proj -> gelu -> down-proj in fp8 DoubleRow
     (weights pre-scaled x64 on host), scales rows by the combine weight
     into a contiguous [128, 9*512] staging tile per H-half, and lands them
     with one dma_scatter_add per half into a zeroed [T, 512] bf16 buffer,
  4. ReduceScatter sums expert contributions across cores (one per H-half);
     the shared expert runs in fp8 DoubleRow for the core's 512-token slice
     (sh_down_b folded into x_slice on the host) and
     out_slice = x_slice' + shared + expert_sum.
Host assembles the 8 slices into the full [B, S, H] output.
"""

import numpy as np
import ml_dtypes

import concourse.bacc as bacc
import concourse.mybir as mybir
import concourse.tile as tile
from concourse import bass
from concourse.bass_utils import run_bass_kernel_spmd
from concourse.masks import make_identity

N_CORES = 8
T = 4096          # tokens
H = 1024          # hidden
F = 4096          # expert hidden
E = 8             # experts
P = 128
TT = T // P       # 32 token tiles
C = 1152          # per-expert token capacity (max actual count ~1086)
CT = C // P       # 9 capacity tiles
SL = T // N_CORES  # 512 tokens owned per core
BIG = 1.0e6       # pad sentinel (> any token id, survives sparse_gather)
WS = 64.0         # fp8 weight scale
NA = 12           # shared-up tiles computed early (before expert phase)

FP = mybir.dt.float32
BF = mybir.dt.bfloat16
F8 = mybir.dt.float8e4
I16 = mybir.dt.int16
DRM = mybir.MatmulPerfMode.DoubleRow
KT = H // P       # 8 contraction tiles
FT = F // P       # 32 expert-hidden tiles


class _NullCtx:
    def __enter__(self):
        return None

    def __exit__(self, *a):
        return False


def build(with_rs=True, loop_n=0, skip_wdma=False, skip_zero=False,
          skip_gather=False, skip_scat=False, skip_sg=False, xt_act=False,
          na2=8, wdeep=True, zlate=False):
    nc = bacc.Bacc("TRN2", target_bir_lowering=False, debug=False,
                   num_devices=N_CORES)

    # ---- I/O ----
    xTr = nc.dram_tensor("xTr", [H, T], BF, kind="ExternalInput")
    xTbs = nc.dram_tensor("xTbs", [H, SL], BF, kind="ExternalInput")
    x_slice = nc.dram_tensor("x_slice", [SL, H], FP, kind="ExternalInput")
    xb8 = nc.dram_tensor("xb8", [T, H], F8, kind="ExternalInput")
    rwb = nc.dram_tensor("rwb", [H, E], BF, kind="ExternalInput")
    rbb = nc.dram_tensor("rbb", [P, E], FP, kind="ExternalInput")
    eselb = nc.dram_tensor("eselb", [P, E], FP, kind="ExternalInput")
    upw8 = nc.dram_tensor("upw8", [F, H], F8, kind="ExternalInput")  # swizzled, xWS
    upb = nc.dram_tensor("upb", [P, FT], FP, kind="ExternalInput")
    dww8 = nc.dram_tensor("dww8", [F, H], F8, kind="ExternalInput")  # xWS
    dwb = nc.dram_tensor("dwb", [1, H], FP, kind="ExternalInput")
    suw = nc.dram_tensor("suw", [F, H], BF, kind="ExternalInput")  # swizzled
    sub = nc.dram_tensor("sub", [P, FT], FP, kind="ExternalInput")
    sdw = nc.dram_tensor("sdw", [F, H], BF, kind="ExternalInput")
    tokid1 = nc.dram_tensor("tokid1", [P, TT], FP, kind="ExternalInput")
    rep16 = nc.dram_tensor("rep16", [16, P], FP, kind="ExternalInput")
    out_slice = nc.dram_tensor("out_slice", [SL, H], FP, kind="ExternalOutput")

    with tile.TileContext(nc) as tc:
        with (
            tc.tile_pool(name="const", bufs=1) as cpool,
            tc.tile_pool(name="sbig", bufs=1) as sbig,
            tc.tile_pool(name="sio", bufs=3) as sio,
            tc.tile_pool(name="wpool", bufs=3) as wpool,
            tc.tile_pool(name="small", bufs=1) as small,
            tc.tile_pool(name="psum", bufs=1, space="PSUM") as psum,
            tc.tile_pool(name="dram", bufs=1, space="DRAM") as dram,
        ):
            # ---- internal DRAM ----
            vvals = dram.tile([P, TT], FP)    # token-or-minus-one, contig
            wvals = dram.tile([P, TT], FP)    # weight-or-minus-one, contig
            wd = dram.tile([C, 1], FP)        # slot-major combine weights
            scatA = dram.tile([T, 512], BF)
            scatB = dram.tile([T, 512], BF)
            rsA = dram.tile([SL, 512], BF)
            rsB = dram.tile([SL, 512], BF)

            # ---- constants ----
            id_b = cpool.tile([P, P], BF)
            make_identity(nc, id_b[:])
            id_8 = cpool.tile([P, P], F8)
            nc.vector.tensor_copy(id_8[:], id_b[:])
            rbb_sb = cpool.tile([P, E], FP)
            nc.sync.dma_start(out=rbb_sb[:], in_=rbb[:])
            eselb_sb = cpool.tile([P, E], FP)
            nc.sync.dma_start(out=eselb_sb[:], in_=eselb[:])
            tok_sb = cpool.tile([P, TT], FP)
            nc.sync.dma_start(out=tok_sb[:], in_=tokid1[:])
            upb_sb = cpool.tile([P, FT], FP)
            nc.sync.dma_start(out=upb_sb[:], in_=upb[:])
            sub_sb = cpool.tile([P, FT], FP)
            nc.sync.dma_start(out=sub_sb[:], in_=sub[:])
            dwb_row = cpool.tile([1, H], FP)
            nc.sync.dma_start(out=dwb_row[:], in_=dwb[:])
            rep16_sb = cpool.tile([16, P], FP)
            nc.sync.dma_start(out=rep16_sb[:], in_=rep16[:])
            ws_row = cpool.tile([1, P], FP)
            nc.vector.memset(ws_row[:], WS)
            zero_big = cpool.tile([P, 1024], BF)
            nc.vector.memset(zero_big[:], 0.0)
            id_f = cpool.tile([P, P], FP)
            make_identity(nc, id_f[:])
            # router weights: [H, E] -> [128, (k e)]
            rw_sb = cpool.tile([P, KT * E], BF)
            nc.sync.dma_start(
                out=rw_sb[:].rearrange("p (k e) -> p k e", e=E),
                in_=rwb[:, :].rearrange("(k p) e -> p k e", p=P))

            # broadcast down-proj bias across partitions via K=1 matmul
            # (scaled by WS to match the fp8-scaled PSUM values)
            dwb_b = cpool.tile([P, H], FP)
            for hck in range(2):
                pb = psum.tile([P, 512], FP, tag="ptp", bufs=1)
                nc.tensor.matmul(
                    out=pb[:], lhsT=ws_row[:],
                    rhs=dwb_row[:, 512 * hck:512 * (hck + 1)],
                    start=True, stop=True)
                nc.vector.tensor_copy(dwb_b[:, 512 * hck:512 * (hck + 1)],
                                      pb[:])

            def body():
                # ---- phase B: local expert-major router (bf16 x bf16) ----
                pt = psum.tile([P, E * TT], FP, tag="ptp", bufs=1)

                def btran(ch, lgc):
                    for jl in range(4):
                        j = 4 * ch + jl
                        nc.tensor.transpose(
                            out=pt[:, E * j:E * (j + 1)],
                            in_=lgc[:, P * jl:P * (jl + 1)],
                            identity=id_f[:E, :E])

                lgcs = []
                for ch in range(KT):
                    xth = []
                    for h2 in range(2):
                        xt = sio.tile([P, 4 * 512], BF, tag="xrt", bufs=3,
                                      name=f"xt{ch}_{h2}")
                        (nc.scalar if xt_act else nc.sync).dma_start(
                            out=xt[:],
                            in_=xTr[P * ch:P * (ch + 1),
                                    2048 * h2:2048 * (h2 + 1)])
                        xth.append(xt)
                    pl = psum.tile([E, 512], FP, tag="pu", bufs=3,
                                   name=f"plr{ch}")
                    for k in range(KT):
                        nc.tensor.matmul(
                            out=pl[:],
                            lhsT=rw_sb[:, E * k:E * (k + 1)],
                            rhs=xth[k // 4][:, 512 * (k % 4):
                                            512 * (k % 4 + 1)],
                            start=(k == 0), stop=(k == KT - 1))
                    lgc = small.tile([E, 512], FP, tag="lgc", bufs=3,
                                     name=f"lgc{ch}")
                    nc.vector.tensor_copy(lgc[:], pl[:])
                    lgcs.append(lgc)
                    if ch >= 2:
                        btran(ch - 2, lgcs[ch - 2])

                # shared-up input; issue early so it lands before G1
                xsh = sbig.tile([P, KT * SL], BF)
                for k in range(KT):
                    nc.scalar.dma_start(out=xsh[:, SL * k:SL * (k + 1)],
                                        in_=xTbs[P * k:P * (k + 1), :])
                # prefetch the residual slice for phase H
                xss = [sbig.tile([P, H], FP, name=f"xss{i}")
                       for i in range(SL // P)]
                for i in range(SL // P):
                    nc.scalar.dma_start(out=xss[i][:],
                                        in_=x_slice[P * i:P * (i + 1), :])

                def zero_scat():
                    # zero the scatter-add targets; 256 rows per DMA
                    if not skip_zero:
                        for j in range(T // 256):
                            for buf in (scatA, scatB):
                                nc.sync.dma_start(
                                    out=buf[256 * j:256 * (j + 1), :]
                                    .rearrange("(p t) c -> p (t c)", t=2),
                                    in_=zero_big[:])

                if not zlate:
                    zero_scat()

                # remaining router transposes
                btran(KT - 2, lgcs[KT - 2])
                btran(KT - 1, lgcs[KT - 1])
                lg = sbig.tile([P, E * TT], FP)
                rbb_bc = rbb_sb[:].rearrange(
                    "p (o e) -> p o e", o=1).to_broadcast([P, TT, E])
                nc.vector.tensor_tensor(
                    out=lg[:].rearrange("p (j e) -> p j e", e=E),
                    in0=pt[:].rearrange("p (j e) -> p j e", e=E),
                    in1=rbb_bc, op=mybir.AluOpType.add)

                # ---- phase C: top-2 mask, my softmax weight (fp32) ----
                lg8 = lg[:].rearrange("p (j e) -> p j e", e=E)
                esel_bc = eselb_sb[:].rearrange(
                    "p (o e) -> p o e", o=1).to_broadcast([P, TT, E])
                sel = small.tile([P, E * TT], FP, bufs=1)
                nc.vector.tensor_tensor(
                    out=sel[:].rearrange("p (j e) -> p j e", e=E),
                    in0=lg8, in1=esel_bc, op=mybir.AluOpType.mult)
                lmy = small.tile([P, TT], FP)
                nc.vector.tensor_reduce(
                    lmy[:], sel[:].rearrange("p (j e) -> p j e", e=E),
                    axis=mybir.AxisListType.X, op=mybir.AluOpType.add)
                m1 = small.tile([P, TT], FP)
                nc.vector.tensor_reduce(m1[:], lg8, axis=mybir.AxisListType.X,
                                        op=mybir.AluOpType.max)
                m1b = m1[:].rearrange("p (j o) -> p j o", o=1).to_broadcast(
                    [P, TT, E])
                lmyb = lmy[:].rearrange("p (j o) -> p j o", o=1).to_broadcast(
                    [P, TT, E])
                gtm = small.tile([P, E * TT], FP, bufs=1)
                nc.vector.tensor_tensor(
                    out=gtm[:].rearrange("p (j e) -> p j e", e=E),
                    in0=lg8, in1=lmyb, op=mybir.AluOpType.is_gt)
                cnt = small.tile([P, TT], FP)
                nc.vector.tensor_reduce(
                    cnt[:], gtm[:].rearrange("p (j e) -> p j e", e=E),
                    axis=mybir.AxisListType.X, op=mybir.AluOpType.add)
                mask0 = small.tile([P, TT], FP)
                nc.vector.tensor_scalar(mask0[:], cnt[:], 1.5, None,
                                        op0=mybir.AluOpType.is_le)
                ex = sel  # sel is dead after lmy; reuse its buffer
                nc.vector.tensor_tensor(
                    out=ex[:].rearrange("p (j e) -> p j e", e=E),
                    in0=lg8, in1=m1b, op=mybir.AluOpType.subtract)
                nc.scalar.activation(ex[:], ex[:],
                                     mybir.ActivationFunctionType.Exp)
                ssum = small.tile([P, TT], FP)
                nc.vector.tensor_reduce(
                    ssum[:], ex[:].rearrange("p (j e) -> p j e", e=E),
                    axis=mybir.AxisListType.X, op=mybir.AluOpType.add)
                rcp = small.tile([P, TT], FP)
                nc.vector.reciprocal(rcp[:], ssum[:])
                tmy = small.tile([P, TT], FP)
                nc.vector.tensor_tensor(out=tmy[:], in0=lmy[:], in1=m1[:],
                                        op=mybir.AluOpType.subtract)
                nc.scalar.activation(tmy[:], tmy[:],
                                     mybir.ActivationFunctionType.Exp)
                w0 = small.tile([P, TT], FP)
                nc.vector.tensor_tensor(out=w0[:], in0=tmy[:], in1=rcp[:],
                                        op=mybir.AluOpType.mult)
                # pre-divide the combine weight by WS (fp8 weight scale)
                nc.vector.tensor_scalar(w0[:], w0[:], 1.0 / WS, None,
                                        op0=mybir.AluOpType.mult)
                # v = tokid1 * mask0 - 1  (token id if selected else -1)
                vv = small.tile([P, TT], FP)
                nc.vector.tensor_tensor(out=vv[:], in0=tok_sb[:], in1=mask0[:],
                                        op=mybir.AluOpType.mult)
                nc.vector.tensor_scalar_add(vv[:], vv[:], -1.0)
                # vw = (w0 + 1) * mask0 - 1  (weight if selected else -1)
                vw = small.tile([P, TT], FP)
                nc.vector.tensor_scalar_add(vw[:], w0[:], 1.0)
                nc.vector.tensor_tensor(out=vw[:], in0=vw[:], in1=mask0[:],
                                        op=mybir.AluOpType.mult)
                nc.vector.tensor_scalar_add(vw[:], vw[:], -1.0)
                # contiguous stores (same bijection for both)
                nc.scalar.dma_start(out=vvals[:, :], in_=vv[:])
                nc.scalar.dma_start(out=wvals[:, :], in_=vw[:])

                # ---- phase D: compact ids + weights via sparse_gather ----
                NPAD = C // 16
                vsb = small.tile([16, T // 16 + NPAD], FP)
                nc.vector.memset(vsb[:], BIG)
                nc.scalar.dma_start(
                    out=vsb[:, :T // 16],
                    in_=vvals[:, :].rearrange("(q r) j -> q (r j)", r=8))
                wsb = small.tile([16, T // 16 + NPAD], FP)
                nc.vector.memset(wsb[:], 0.0)  # pad weights -> 0
                nc.scalar.dma_start(
                    out=wsb[:, :T // 16],
                    in_=wvals[:, :].rearrange("(q r) j -> q (r j)", r=8))
                gout = small.tile([16, C // 16], FP)
                wout = small.tile([16, C // 16], FP)
                if skip_sg:
                    nc.vector.memset(gout[:], 5.0)
                    nc.vector.memset(wout[:], 0.001)
                else:
                    ng = small.tile([1, 1], mybir.dt.uint32)
                    nc.gpsimd.sparse_gather(out=gout[:], in_=vsb[:],
                                            num_found=ng[:])
                    ng2 = small.tile([1, 1], mybir.dt.uint32)
                    nc.gpsimd.sparse_gather(out=wout[:], in_=wsb[:],
                                            num_found=ng2[:])
                # index values: pads -> 0 for the gather, -1 for the scatter
                selm = small.tile([16, C // 16], FP)
                nc.vector.tensor_scalar(selm[:], gout[:], float(T), None,
                                        op0=mybir.AluOpType.is_lt)
                idgf = small.tile([16, C // 16], FP)
                nc.vector.tensor_tensor(out=idgf[:], in0=gout[:], in1=selm[:],
                                        op=mybir.AluOpType.mult)
                # slot-major combine weights -> [P, CT]
                nc.scalar.dma_start(
                    out=wd[:, 0].rearrange("(f q) -> q f", q=16), in_=wout[:])
                wc_all = cpool.tile([P, CT], FP, name="wc_all")
                nc.scalar.dma_start(
                    out=wc_all[:],
                    in_=wd[:, 0].rearrange("(i p) -> p i", p=P))

                # ---- phase G1: shared expert up-proj (bf16), NA tiles ----
                sgt = sbig.tile([P, FT * SL], BF)

                def shared_up(ft):
                    pu = psum.tile([P, 512], FP, tag="pu", bufs=3,
                                   name=f"psh{ft}")
                    uw = wpool.tile([P, KT * P], BF, tag="suw",
                                    bufs=5 if wdeep else 4,
                                    name=f"suw{ft}")
                    nc.sync.dma_start(out=uw[:],
                                      in_=suw[P * ft:P * (ft + 1), :])
                    for k in range(KT):
                        nc.tensor.matmul(out=pu[:],
                                         lhsT=uw[:, P * k:P * (k + 1)],
                                         rhs=xsh[:, SL * k:SL * (k + 1)],
                                         start=(k == 0), stop=(k == KT - 1))
                    nc.scalar.activation(
                        sgt[:, SL * ft:SL * (ft + 1)], pu[:],
                        mybir.ActivationFunctionType.Gelu,
                        bias=sub_sb[:, ft:ft + 1])

                for ft in range(4):
                    shared_up(ft)

                # replicate idx values to all 8 gpsimd core groups (the SWDGE
                # ucode reads indexes from its own 16 partitions) via a
                # one-hot [16, 128] broadcast matmul, then convert to int16.
                # pads point at token 0; their rows carry weight 0 so the
                # scatter-add contributes nothing.
                idx16g = small.tile([P, C // 16], I16, bufs=1)
                pr = psum.tile([P, C // 16], FP, tag="ptp", bufs=1,
                               name="pr_g")
                nc.tensor.matmul(out=pr[:], lhsT=rep16_sb[:],
                                 rhs=idgf[:], start=True, stop=True)
                nc.vector.tensor_copy(idx16g[:], pr[:])

                # ---- phase E: bulk-gather token rows, transpose to fp8 ----
                # (split at 512 slots: the SWDGE ring holds 1024 descriptors;
                # issued now so the gather runs under the rest of G1)
                xg8 = sbig.tile([P, CT * H], F8)
                xg8sp = xg8[:].rearrange("p (i h) -> p i h", h=H)
                if skip_gather:
                    for i in range(CT):
                        nc.sync.dma_start(out=xg8sp[:, i, :],
                                          in_=xb8[P * i:P * (i + 1), :])
                else:
                    nc.gpsimd.dma_gather(
                        out_ap=xg8sp[:, :4, :],
                        in_ap=xb8[:, :], idxs_ap=idx16g[:, :32],
                        num_idxs=512, num_idxs_reg=512, elem_size=H)
                    nc.gpsimd.dma_gather(
                        out_ap=xg8sp[:, 4:, :],
                        in_ap=xb8[:, :], idxs_ap=idx16g[:, 32:],
                        num_idxs=C - 512, num_idxs_reg=C - 512, elem_size=H)

                if zlate:
                    zero_scat()
                for ft in range(4, 4 + na2):
                    shared_up(ft)

                xg8v = xg8[:].rearrange("p (i h) -> p i h", h=H)
                xcT8 = sbig.tile([P, KT * C], F8)
                for i in range(CT):
                    xgb = sio.tile([P, H], BF, tag="xgb", bufs=3,
                                   name=f"xgb{i}")
                    nc.scalar.activation(xgb[:], xg8v[:, i, :],
                                         mybir.ActivationFunctionType.Copy)
                    px = psum.tile([P, KT * P], BF, tag="pu", bufs=3,
                                   name=f"px{i}")
                    for k in range(KT):
                        nc.tensor.transpose(out=px[:, P * k:P * (k + 1)],
                                            in_=xgb[:, P * k:P * (k + 1)],
                                            identity=id_b[:])
                    nc.vector.tensor_copy(
                        xcT8[:].rearrange("p (k c) -> p k c", c=C)[
                            :, :, P * i:P * (i + 1)],
                        px[:].rearrange("p (k c) -> p k c", c=P))

                # ---- phase F: fp8 DoubleRow up-proj -> gelu -> down-proj
                # -> staged scatter-add, H-half-major for early RS ----
                ugt8 = sbig.tile([P, FT * C], F8)
                xcT8v = xcT8[:].rearrange("p (k c) -> p k c", c=C)
                for ft in range(FT):
                    uw8 = wpool.tile([P, KT * P], F8, tag="uw8",
                                      bufs=6 if wdeep else 4,
                                      name=f"uw{ft}")
                    if skip_wdma:
                        nc.vector.memset(uw8[:], 0.5)
                    else:
                        nc.sync.dma_start(
                            out=uw8[:], in_=upw8[P * ft:P * (ft + 1), :])
                    uw8v = uw8[:].rearrange("p (k q) -> p k q", q=P)
                    CH3 = ((0, 512), (512, 512), (1024, C - 1024))
                    pus = [psum.tile([P, 512], FP, tag="pu", bufs=3,
                                     name=f"pu{ft}_{soff}")
                           if c3 < 2 else
                           psum.tile([P, 512], FP, tag="ptp", bufs=1,
                                     name=f"pu{ft}_{soff}")
                           for c3, (soff, slen) in enumerate(CH3)]
                    # kp outer so each DoubleRow LDWEIGHTS serves all three
                    # column chunks (4 LDWs per ft instead of 12)
                    for kp in range(KT // 2):
                        for c3, (soff, slen) in enumerate(CH3):
                            nc.tensor.matmul(
                                out=pus[c3][:, :slen],
                                lhsT=uw8v[:, 2 * kp:2 * kp + 2, :],
                                rhs=xcT8v[:, 2 * kp:2 * kp + 2,
                                          soff:soff + slen],
                                start=(kp == 0), stop=(kp == KT // 2 - 1),
                                perf_mode=DRM)
                    for c3, (soff, slen) in enumerate(CH3):
                        nc.scalar.activation(
                            ugt8[:, C * ft + soff:C * ft + soff + slen],
                            pus[c3][:, :slen],
                            mybir.ActivationFunctionType.Gelu,
                            scale=1.0 / WS, bias=upb_sb[:, ft:ft + 1])

                ugt8v = ugt8[:].rearrange("p (f c) -> p f c", c=C)
                gap_ft = [4 + na2]  # shared-up tiles slotted into drains
                ysb = sbig.tile([P, CT * 512], BF, name="ysb")
                for hc in range(2):
                    for (b0, b1) in ((0, 4), (4, CT)):
                        # batch 2 carries 5 tiles: 4 on the pd ring plus one
                        # on the otherwise-idle ptp bank
                        pds = [psum.tile([P, 512], FP, tag="pd", bufs=4,
                                         name=f"pd{hc}_{i}")
                               if i < b0 + 4 else
                               psum.tile([P, 512], FP, tag="ptp", bufs=1,
                                         name=f"pd{hc}_{i}")
                               for i in range(b0, b1)]
                        for fp_ in range(FT // 2):
                            dw2 = wpool.tile([P, 2 * 512], F8, tag="dw8",
                                             bufs=8 if wdeep else 6,
                                             name=f"dw{hc}_{fp_}")
                            if skip_wdma:
                                nc.vector.memset(dw2[:], 0.5)
                            else:
                                nc.sync.dma_start(
                                    out=dw2[:].rearrange(
                                        "p (two h) -> p two h", two=2),
                                    in_=dww8[256 * fp_:256 * (fp_ + 1),
                                             512 * hc:512 * (hc + 1)]
                                    .rearrange("(two p) h -> p two h", p=P))
                            dw2v = dw2[:].rearrange("p (two h) -> p two h",
                                                    two=2)
                            for i in range(b0, b1):
                                nc.tensor.matmul(
                                    out=pds[i - b0][:],
                                    lhsT=ugt8v[:, 2 * fp_:2 * fp_ + 2,
                                               P * i:P * (i + 1)],
                                    rhs=dw2v,
                                    start=(fp_ == 0),
                                    stop=(fp_ == FT // 2 - 1),
                                    perf_mode=DRM)
                        for i in range(b0, b1):
                            ys = ysb[:, 512 * i:512 * (i + 1)]
                            nc.vector.tensor_add(
                                ys, pds[i - b0][:],
                                dwb_b[:, 512 * hc:512 * (hc + 1)])
                            nc.vector.tensor_scalar_mul(
                                ys, ys, wc_all[:, i:i + 1])
                        # independent PE work to cover the batch's
                        # accumulation-group drain
                        shared_up(gap_ft[0])
                        gap_ft[0] += 1
                        # scatter this batch while the next one computes
                        scat = scatA if hc == 0 else scatB
                        ysbv = ysb[:].rearrange("p (i c) -> p i c", c=512)
                        if skip_scat:
                            for i in range(b0, b1):
                                if P * (i + 1) <= T:
                                    nc.sync.dma_start(
                                        out=scat[P * i:P * (i + 1), :],
                                        in_=ysbv[:, i, :])
                        elif b0 == 0:
                            nc.gpsimd.dma_scatter_add(
                                scat[:, :], ysbv[:, :4, :],
                                idx16g[:, :32], 512, 512, 512)
                        else:
                            nc.gpsimd.dma_scatter_add(
                                scat[:, :], ysbv[:, 4:, :],
                                idx16g[:, 32:], C - 512, C - 512, 512)
                    src, dst = (scatA, rsA) if hc == 0 else (scatB, rsB)
                    if with_rs:
                        nc.gpsimd.collective_compute(
                            "ReduceScatter", mybir.AluOpType.add,
                            replica_groups=[list(range(N_CORES))],
                            ins=[src[:]], outs=[dst[:]])
                    else:
                        nc.sync.dma_start(out=dst[:], in_=src[:SL, :])

                # ---- phase G2: rest of shared expert (hides the RS tail)
                for ft in range(gap_ft[0], FT):
                    shared_up(ft)
                sho = [sbig.tile([P, H], BF, name=f"sho{i}")
                       for i in range(SL // P)]
                for hc in range(2):
                    pdsh = [psum.tile([P, 512], FP, tag="pd", bufs=4,
                                      name=f"pds{hc}_{i}")
                            for i in range(SL // P)]
                    for ft in range(FT):
                        dwt = wpool.tile([P, 512], BF, tag="sdw",
                                         bufs=8 if wdeep else 6,
                                         name=f"sdw{hc}_{ft}")
                        nc.sync.dma_start(
                            out=dwt[:],
                            in_=sdw[P * ft:P * (ft + 1),
                                    512 * hc:512 * (hc + 1)])
                        for i in range(SL // P):
                            nc.tensor.matmul(
                                out=pdsh[i][:],
                                lhsT=sgt[:, SL * ft + P * i:
                                         SL * ft + P * (i + 1)],
                                rhs=dwt[:],
                                start=(ft == 0), stop=(ft == FT - 1))
                    for i in range(SL // P):
                        nc.vector.tensor_copy(
                            sho[i][:, 512 * hc:512 * (hc + 1)],
                            pdsh[i][:])

                # ---- phase H: out = x_slice' + shared + expert sum ----
                for i in range(SL // P):
                    rsl = sio.tile([P, H], BF, tag="rsl", bufs=2)
                    nc.sync.dma_start(out=rsl[:, :512],
                                      in_=rsA[P * i:P * (i + 1), :])
                    nc.sync.dma_start(out=rsl[:, 512:],
                                      in_=rsB[P * i:P * (i + 1), :])
                    nc.vector.tensor_add(xss[i][:], xss[i][:], rsl[:])
                    nc.vector.tensor_add(xss[i][:], xss[i][:], sho[i][:])
                    nc.sync.dma_start(out=out_slice[P * i:P * (i + 1), :],
                                      in_=xss[i][:])

            with (tc.For_i(0, loop_n, 1) if loop_n else _NullCtx()):
                body()
    nc.finalize()
    return nc


_NC_CACHE = None


def _get_nc():
    global _NC_CACHE
    if _NC_CACHE is None:
        _NC_CACHE = build()
    return _NC_CACHE


def _swizzle(w):
    # [H, F] -> [F, H] with row = ft*128 + h%128, col = (h//128)*128 + f%128
    return np.ascontiguousarray(
        w.reshape(H // P, P, F // P, P).transpose(2, 1, 0, 3).reshape(F, H))


def make_in_maps(inputs):
    x = np.asarray(inputs["hidden_states"], dtype=np.float32).reshape(T, H)
    router_w = np.asarray(inputs["router_w"], dtype=np.float32)
    router_b = np.asarray(inputs["router_b"], dtype=np.float32)
    up_w = np.asarray(inputs["up_w"], dtype=np.float32)
    up_b = np.asarray(inputs["up_b"], dtype=np.float32)
    down_w = np.asarray(inputs["down_w"], dtype=np.float32)
    down_b = np.asarray(inputs["down_b"], dtype=np.float32)
    sh_up_w = np.asarray(inputs["sh_up_w"], dtype=np.float32)
    sh_up_b = np.asarray(inputs["sh_up_b"], dtype=np.float32)
    sh_down_w = np.asarray(inputs["sh_down_w"], dtype=np.float32)
    sh_down_b = np.asarray(inputs["sh_down_b"], dtype=np.float32)

    bf = ml_dtypes.bfloat16
    f8 = ml_dtypes.float8_e4m3

    def q8(a):
        return np.ascontiguousarray(np.clip(a * WS, -240, 240).astype(f8))

    def q8x(a):
        return np.ascontiguousarray(np.clip(a, -240, 240).astype(f8))

    xT = np.ascontiguousarray(x.T)
    # router chunk-contiguous layout
    xTr_ = np.ascontiguousarray(
        xT.reshape(KT, P, T // 512, 512).transpose(2, 1, 0, 3)
        .reshape(H, T).astype(bf))
    xb8_ = q8x(x)
    rwb_ = np.ascontiguousarray(router_w.astype(bf))
    rbb_ = np.ascontiguousarray(np.tile(router_b.reshape(1, E), (P, 1)))
    tokid1 = (np.arange(P)[:, None] + P * np.arange(TT)[None, :] + 1.0).astype(
        np.float32)
    suw_ = np.ascontiguousarray(_swizzle(sh_up_w).astype(bf))
    sub_ = np.ascontiguousarray(sh_up_b.reshape(FT, P).T.astype(np.float32))
    sdw_ = np.ascontiguousarray(sh_down_w.astype(bf))
    eye = np.eye(E, dtype=np.float32)
    rep16_ = np.ascontiguousarray(
        np.tile(np.eye(16, dtype=np.float32), (1, P // 16)).reshape(16, P))

    in_maps = []
    for c in range(N_CORES):
        in_maps.append({
            "xTr": xTr_,
            "xTbs": np.ascontiguousarray(
                xT[:, SL * c:SL * (c + 1)].astype(bf)),
            "x_slice": np.ascontiguousarray(
                x[SL * c:SL * (c + 1)] + sh_down_b.reshape(1, H)),
            "xb8": xb8_,
            "rwb": rwb_,
            "rbb": rbb_,
            "eselb": np.ascontiguousarray(np.tile(eye[c:c + 1], (P, 1))),
            "upw8": q8(_swizzle(up_w[c])),
            "upb": np.ascontiguousarray(
                up_b[c].reshape(FT, P).T.astype(np.float32)),
            "dww8": q8(down_w[c]),
            "dwb": down_b[c].reshape(1, H).astype(np.float32),
            "suw": suw_, "sub": sub_, "sdw": sdw_,
            "tokid1": tokid1, "rep16": rep16_,
        })
    return in_maps


def assemble(results):
    out = np.concatenate([results[c]["out_slice"] for c in range(N_CORES)],
                         axis=0)
    return out.reshape(2, 2048, H).astype(np.float32)


def kernel(**inputs):
    nc = _get_nc()
    in_maps = make_in_maps(inputs)
    res = run_bass_kernel_spmd(nc, in_maps, core_ids=list(range(N_CORES)))
    return assemble(res.results)
